# revision 8
# baseline (speedup 1.0000x reference)
"""Trainium2 Bass kernel for the ragged-sequence LSTM encoder.

Math: masked LSTM over T=64 steps, B=16384, E=64, H=128. Reference:
  mask[t,b] = ~isnan(obs[t,b,0]); x = nan_to_num(obs)
  emb = x @ W_emb + b_emb
  gates = emb_t @ w_ih.T + h @ w_hh.T + (b_ih + b_hh);  i,f,g,o
  c' = f*c + i*g ; h' = o*tanh(c'); carry updated only where mask.

Kernel reformulation (exact up to fp rounding):
- The NaN prefix is monotone (invalid iff t < start[b]), so masked lanes
  have h=c=0 until their first valid step. Forcing i=0 and o=0 on masked
  lanes keeps h=c=0 exactly -> no select/blend ops needed. Done by an
  extra "nan indicator" input row with weight -BIG on the i/o gate
  blocks (sigmoid saturates to 0 exactly).
- Embedding folded into the input weights: W_x = W_emb @ w_ih.T,
  b_x = b_emb @ w_ih.T + b_ih + b_hh (both computed on device). Per-step
  input is x~_t = [x0, x1, 1, nan_ind, 0...] zero-padded to K=128 --
  the pad costs no PE cycles (cost ~ N columns) and keeps every matmul
  at the full (128,128) stationary shape; interleaving K=4 LDWEIGHTS
  with K=128 ones was measured to break PE pipelining (535 vs 216
  ns/matmul).
- Layout: hidden/gate dim on partitions, batch on free dim. Batch in 4
  chunks of 512 (PSUM bank per gate block). PSUM gate order [i,f,o,g]
  so one sigmoid ACTIVATE covers i,f,o and one tanh covers g.
- Per chunk per step:  psum = x~ @ W~ + H @ WhhT   (8 matmuls, K=128)
    sig = sigmoid(psum[:, :3C]); tg = tanh(psum[:, 3C:])
    ig = i*tg; fc = f*c; c' = ig + fc; th = tanh(c'); h' = o*th
  (4 fp16 tensor_tensor DVE ops at 2x mode + 3 ACT ops)
- Data parallel over batch: core k takes columns [k*2048,(k+1)*2048).
  Weights replicated.

Host code only reshapes/shards/transposes and supplies constants; all
FLOPs (weight fusion, NaN handling, LSTM) run on device.
"""

import sys
import numpy as np

for _p in ("/opt/trn_rl_repo", "/root/.axon_site/_ro/trn_rl_repo"):
    if _p not in sys.path:
        sys.path.insert(0, _p)

import concourse.bacc as bacc
import concourse.tile as tile
import concourse.mybir as mybir
from concourse.bass_utils import run_bass_kernel_spmd

F32 = mybir.dt.float32
F16 = mybir.dt.float16
AOP = mybir.AluOpType
ACTF = mybir.ActivationFunctionType

N_CORES = 8
T = 64
B = 16384
E = 64
H = 128
BL = B // N_CORES          # 2048 batch per core
C = 512                    # batch chunk (one PSUM bank per gate block)
NCHUNK = BL // C           # 4
BLK = 8                    # time steps per streamed x~ block
NBLK = T // BLK
BIG = 30000.0


def _build_program(widths):
    nc = bacc.Bacc()

    obs_p = nc.dram_tensor("obs_p", [2 * T, BL], F32, kind="ExternalInput")
    wemb3 = nc.dram_tensor("wemb3", [E, 3], F32, kind="ExternalInput")
    wihT = nc.dram_tensor("wihT", [E, 4 * H], F32, kind="ExternalInput")
    b2 = nc.dram_tensor("b2", [2, 4 * H], F32, kind="ExternalInput")
    sel23 = nc.dram_tensor("sel23", [2, 3], F32, kind="ExternalInput")
    whhT = nc.dram_tensor("whhT", [H, 4 * H], F32, kind="ExternalInput")
    mask16 = nc.dram_tensor("mask16", [1, 4 * H], F16, kind="ExternalInput")
    ones16 = nc.dram_tensor("ones16", [1, BLK * BL], F16, kind="ExternalInput")
    h_out = nc.dram_tensor("h_out", [H, BL], F32, kind="ExternalOutput")

    with tile.TileContext(nc) as tc:
        with (
            tc.tile_pool(name="const", bufs=1) as cp,
            tc.tile_pool(name="work", bufs=6) as wp,
        ):
            # ---- one-time prep ----
            obs_sb = cp.tile([2 * T, BL], F32, name="obs_sb")
            nc.sync.dma_start(out=obs_sb[:], in_=obs_p[:])
            wemb3_sb = cp.tile([E, 3], F32, name="wemb3_sb")
            nc.sync.dma_start(out=wemb3_sb[:], in_=wemb3[:])
            wihT_sb = cp.tile([E, 4 * H], F32, name="wihT_sb")
            nc.sync.dma_start(out=wihT_sb[:], in_=wihT[:])
            b2_sb = cp.tile([2, 4 * H], F32, name="b2_sb")
            nc.sync.dma_start(out=b2_sb[:], in_=b2[:])
            sel23_sb = cp.tile([2, 3], F32, name="sel23_sb")
            nc.sync.dma_start(out=sel23_sb[:], in_=sel23[:])
            whhT_sb = cp.tile([H, 4 * H], F32, name="whhT_sb")
            nc.sync.dma_start(out=whhT_sb[:], in_=whhT[:])

            # NaN indicator (1.0 where NaN) per (t,b), per channel half
            ind = cp.tile([2 * T, BL], F16, name="ind")
            nc.vector.tensor_tensor(ind[0:T, :], obs_sb[0:T, :], obs_sb[0:T, :],
                                    AOP.not_equal)
            nc.vector.tensor_tensor(ind[T:2 * T, :], obs_sb[T:2 * T, :],
                                    obs_sb[T:2 * T, :], AOP.not_equal)
            # cleaned fp16 obs: NaN -> 0
            zeros = cp.tile([2 * T, BL], F16, name="zeros")
            nc.vector.memset(zeros[:], 0.0)
            obs16 = cp.tile([2 * T, BL], F16, name="obs16")
            nc.vector.tensor_copy(obs16[:], obs_sb[:])
            nc.vector.copy_predicated(obs16[:], ind[:].bitcast(mybir.dt.uint16),
                                      zeros[:])

            # fused input weights: psum_w = [W_x0; W_x1; b_x] (3, 512),
            # torch gate order i,f,g,o
            wt16 = cp.tile([H, 4 * H], F16, name="wt16")
            nc.vector.memset(wt16[:], 0.0)
            with tc.tile_pool(name="psum_prep", bufs=1, space="PSUM") as pp:
                psum_w = pp.tile([3, 4 * H], F32, name="psum_w")
                nc.tensor.matmul(psum_w[:], wemb3_sb[:], wihT_sb[:],
                                 start=True, stop=False)
                nc.tensor.matmul(psum_w[:], sel23_sb[:], b2_sb[:],
                                 start=False, stop=True)
                # W~ fp16 (128, 512) zero-padded; gate column order i,f,o,g
                nc.vector.tensor_copy(wt16[0:3, 0:2 * H], psum_w[:, 0:2 * H])
                nc.vector.tensor_copy(wt16[0:3, 2 * H:3 * H],
                                      psum_w[:, 3 * H:4 * H])
                nc.vector.tensor_copy(wt16[0:3, 3 * H:4 * H],
                                      psum_w[:, 2 * H:3 * H])
                nc.sync.dma_start(out=wt16[3:4, :], in_=mask16[:])

            # WhhT fp16, gate column order i,f,o,g
            whh16 = cp.tile([H, 4 * H], F16, name="whh16")
            nc.vector.tensor_copy(whh16[:, 0:2 * H], whhT_sb[:, 0:2 * H])
            nc.vector.tensor_copy(whh16[:, 2 * H:3 * H], whhT_sb[:, 3 * H:4 * H])
            nc.vector.tensor_copy(whh16[:, 3 * H:4 * H], whhT_sb[:, 2 * H:3 * H])

            # state (true scale)
            Hs = cp.tile([H, BL], F16, name="Hs")
            Cs = cp.tile([H, BL], F16, name="Cs")
            nc.vector.memset(Hs[:], 0.0)
            nc.vector.memset(Cs[:], 0.0)

            # x~ ping-pong buffers, zero-padded to K=128 once
            xbufs = []
            for i in range(2):
                xb = cp.tile([H, BLK * BL], F16, name=f"xb{i}")
                nc.vector.memset(xb[:], 0.0)
                xbufs.append(xb)

            # ---- steps (ragged: only the valid prefix width per step) ----
            with tc.tile_pool(name="psum_gates", bufs=2, space="PSUM") as gp:
                for tb in range(NBLK):
                    xb = xbufs[tb % 2]
                    t0 = tb * BLK
                    nc.sync.dma_start(out=xb[0:1, :], in_=obs16[t0:t0 + BLK, :])
                    nc.sync.dma_start(out=xb[1:2, :],
                                      in_=obs16[T + t0:T + t0 + BLK, :])
                    nc.sync.dma_start(out=xb[2:3, :], in_=ones16[:])
                    nc.sync.dma_start(out=xb[3:4, :], in_=ind[t0:t0 + BLK, :])

                    for dt_ in range(BLK):
                        t = t0 + dt_
                        W = widths[t]
                        nchunk = (W + C - 1) // C
                        sigs = []
                        for j in range(nchunk):
                            cw = min(C, W - j * C)
                            jc = slice(j * C, j * C + cw)
                            xoff = dt_ * BL + j * C
                            rhs_x = xb[:, xoff:xoff + cw]
                            g_ps = gp.tile([H, 4 * C], F32, name="g_ps")
                            for pb in range(4):
                                gs = slice(pb * C, pb * C + cw)
                                wsl = slice(pb * H, (pb + 1) * H)
                                nc.tensor.matmul(g_ps[:, gs], wt16[:, wsl],
                                                 rhs_x, start=True, stop=False)
                                nc.tensor.matmul(g_ps[:, gs], whh16[:, wsl],
                                                 Hs[:, jc], start=False,
                                                 stop=True)
                            sig = wp.tile([H, 3 * C], F16, name="sig")
                            if cw == C:
                                nc.scalar.activation(sig[:], g_ps[:, 0:3 * C],
                                                     ACTF.Sigmoid)
                            else:
                                sig_src = g_ps[:, 0:3 * C].rearrange(
                                    "p (g c) -> p g c", g=3)[:, :, 0:cw]
                                nc.scalar.activation(
                                    sig[:, 0:3 * cw].rearrange(
                                        "p (g c) -> p g c", g=3),
                                    sig_src, ACTF.Sigmoid)
                            tgg = wp.tile([H, C], F16, name="tgg")
                            nc.scalar.activation(tgg[:, 0:cw],
                                                 g_ps[:, 3 * C:3 * C + cw],
                                                 ACTF.Tanh)
                            ig = wp.tile([H, C], F16, name="ig")
                            nc.vector.tensor_tensor(ig[:, 0:cw],
                                                    sig[:, 0:cw], tgg[:, 0:cw],
                                                    AOP.mult)
                            fc = wp.tile([H, C], F16, name="fc")
                            nc.vector.tensor_tensor(fc[:, 0:cw],
                                                    sig[:, cw:2 * cw],
                                                    Cs[:, jc], AOP.mult)
                            nc.vector.tensor_tensor(Cs[:, jc], ig[:, 0:cw],
                                                    fc[:, 0:cw], AOP.add)
                            sigs.append((j, cw, sig))
                        th = wp.tile([H, BL], F16, name="th")
                        nc.scalar.activation(th[:, 0:W], Cs[:, 0:W], ACTF.Tanh)
                        for j, cw, sig in sigs:
                            jc = slice(j * C, j * C + cw)
                            nc.vector.tensor_tensor(Hs[:, jc],
                                                    sig[:, 2 * cw:3 * cw],
                                                    th[:, jc], AOP.mult)

            # ---- output (fp32) ----
            hout = cp.tile([H, BL], F32, name="hout")
            nc.vector.tensor_copy(hout[:], Hs[:])
            nc.sync.dma_start(out=h_out[:], in_=hout[:])

    nc.compile()
    return nc


_CACHE = {}


def _plan(obs_traj):
    """Sort batch by ragged start (sharding permutation) and derive the
    per-step valid prefix width each core must process. Any width >= the
    true valid count is correct (masked lanes stay exactly 0)."""
    obs_traj = np.asarray(obs_traj)
    start = np.isnan(obs_traj[:, :, 0]).sum(0)          # (B,)
    perm = np.argsort(start, kind="stable")
    start_sorted = start[perm]
    ts = np.arange(T)
    vglob = np.searchsorted(start_sorted, ts, side="right")  # valid count
    w = np.ceil(vglob / N_CORES).astype(np.int64)
    w = np.minimum(BL, ((w + 7) // 8) * 8)
    w = np.maximum(w, 8)
    return perm, tuple(int(x) for x in w)


def _host_inputs(obs_traj, W_emb, b_emb, w_ih, w_hh, b_ih, b_hh, perm):
    f32 = np.float32
    wemb3 = np.concatenate(
        [np.asarray(W_emb, f32).T, np.asarray(b_emb, f32)[:, None]], axis=1
    )  # (64, 3)
    wihT = np.ascontiguousarray(np.asarray(w_ih, f32).T)      # (64, 512)
    whhT = np.ascontiguousarray(np.asarray(w_hh, f32).T)      # (128, 512)
    b2 = np.ascontiguousarray(
        np.stack([np.asarray(b_ih, f32), np.asarray(b_hh, f32)], axis=0)
    )  # (2, 512)
    sel23 = np.array([[0, 0, 1], [0, 0, 1]], f32)             # (2, 3)
    # mask row in device gate order [i, f, o, g]
    maskrow = np.zeros((1, 4 * H), np.float16)
    maskrow[0, 0:H] = -BIG          # i
    maskrow[0, 2 * H:3 * H] = -BIG  # o
    ones16 = np.ones((1, BLK * BL), np.float16)

    obs_traj = np.asarray(obs_traj)
    in_maps = []
    for k in range(N_CORES):
        sl = np.asarray(obs_traj[:, perm[k::N_CORES], :], f32)  # (T, BL, 2)
        obs_p = np.ascontiguousarray(
            sl.transpose(2, 0, 1).reshape(2 * T, BL)
        )  # (128, BL): row f*T + t
        in_maps.append({
            "obs_p": obs_p, "wemb3": wemb3, "wihT": wihT, "b2": b2,
            "sel23": sel23, "whhT": whhT, "mask16": maskrow, "ones16": ones16,
        })
    return in_maps


def kernel(obs_traj, W_emb, b_emb, w_ih, w_hh, b_ih, b_hh):
    perm, widths = _plan(obs_traj)
    if _CACHE.get("widths") != widths:
        _CACHE["nc"] = _build_program(widths)
        _CACHE["widths"] = widths
    nc = _CACHE["nc"]

    in_maps = _host_inputs(obs_traj, W_emb, b_emb, w_ih, w_hh, b_ih, b_hh,
                           perm)
    res = run_bass_kernel_spmd(nc, in_maps, list(range(N_CORES)))

    out = np.empty((1, B, H), np.float32)
    for k in range(N_CORES):
        out[0, perm[k::N_CORES], :] = res.results[k]["h_out"].T
    return out


# revision 9
# speedup vs baseline: 1.4976x; 1.4976x over previous
"""Trainium2 Bass kernel for the ragged-sequence LSTM encoder.

Math: masked LSTM over T=64 steps, B=16384, E=64, H=128. Reference:
  mask[t,b] = ~isnan(obs[t,b,0]); x = nan_to_num(obs)
  emb = x @ W_emb + b_emb
  gates = emb_t @ w_ih.T + h @ w_hh.T + (b_ih + b_hh);  i,f,g,o
  c' = f*c + i*g ; h' = o*tanh(c'); carry updated only where mask.

Kernel reformulation (exact up to fp rounding):
- The NaN prefix is monotone (invalid iff t < start[b]), so masked lanes
  have h=c=0 until their first valid step. Forcing i=0 and o=0 on masked
  lanes keeps h=c=0 exactly -> no select/blend ops needed. Done by an
  extra "nan indicator" input row with weight -BIG on the i/o gate
  blocks (sigmoid saturates to 0 exactly).
- Embedding folded into the input weights: W_x = W_emb @ w_ih.T,
  b_x = b_emb @ w_ih.T + b_ih + b_hh (both computed on device). Per-step
  input is x~_t = [x0, x1, 1, nan_ind, 0...] zero-padded to K=128 --
  the pad costs no PE cycles (cost ~ N columns) and keeps every matmul
  at the full (128,128) stationary shape; interleaving K=4 LDWEIGHTS
  with K=128 ones was measured to break PE pipelining (535 vs 216
  ns/matmul).
- Layout: hidden/gate dim on partitions, batch on free dim. Batch in 4
  chunks of 512 (PSUM bank per gate block). PSUM gate order [i,f,o,g]
  so one sigmoid ACTIVATE covers i,f,o and one tanh covers g.
- Per chunk per step:  psum = x~ @ W~ + H @ WhhT   (8 matmuls, K=128)
    sig = sigmoid(psum[:, :3C]); tg = tanh(psum[:, 3C:])
    ig = i*tg; fc = f*c; c' = ig + fc; th = tanh(c'); h' = o*th
  (4 fp16 tensor_tensor DVE ops at 2x mode + 3 ACT ops)
- Data parallel over batch: core k takes columns [k*2048,(k+1)*2048).
  Weights replicated.

Host code only reshapes/shards/transposes and supplies constants; all
FLOPs (weight fusion, NaN handling, LSTM) run on device.
"""

import sys
import numpy as np

for _p in ("/opt/trn_rl_repo", "/root/.axon_site/_ro/trn_rl_repo"):
    if _p not in sys.path:
        sys.path.insert(0, _p)

import concourse.bacc as bacc
import concourse.tile as tile
import concourse.mybir as mybir
from concourse.bass_utils import run_bass_kernel_spmd

F32 = mybir.dt.float32
F16 = mybir.dt.float16
AOP = mybir.AluOpType
ACTF = mybir.ActivationFunctionType

N_CORES = 8
T = 64
B = 16384
E = 64
H = 128
BL = B // N_CORES          # 2048 batch per core
C = 512                    # batch chunk (one PSUM bank per gate block)
NCHUNK = BL // C           # 4
BLK = 8                    # time steps per streamed x~ block
NBLK = T // BLK
BIG = 30000.0


def _build_program(widths):
    nc = bacc.Bacc()

    obs_p = nc.dram_tensor("obs_p", [2 * T, BL], F32, kind="ExternalInput")
    wemb3 = nc.dram_tensor("wemb3", [E, 3], F32, kind="ExternalInput")
    wihT = nc.dram_tensor("wihT", [E, 4 * H], F32, kind="ExternalInput")
    b2 = nc.dram_tensor("b2", [2, 4 * H], F32, kind="ExternalInput")
    sel23 = nc.dram_tensor("sel23", [2, 3], F32, kind="ExternalInput")
    whhT = nc.dram_tensor("whhT", [H, 4 * H], F32, kind="ExternalInput")
    mask16 = nc.dram_tensor("mask16", [1, 4 * H], F16, kind="ExternalInput")
    ones16 = nc.dram_tensor("ones16", [1, BLK * BL], F16, kind="ExternalInput")
    h_out = nc.dram_tensor("h_out", [H, BL], F32, kind="ExternalOutput")

    with tile.TileContext(nc) as tc:
        with (
            tc.tile_pool(name="const", bufs=1) as cp,
            tc.tile_pool(name="work", bufs=6) as wp,
        ):
            # ---- one-time prep ----
            obs_sb = cp.tile([2 * T, BL], F32, name="obs_sb")
            nc.sync.dma_start(out=obs_sb[:], in_=obs_p[:])
            wemb3_sb = cp.tile([E, 3], F32, name="wemb3_sb")
            nc.sync.dma_start(out=wemb3_sb[:], in_=wemb3[:])
            wihT_sb = cp.tile([E, 4 * H], F32, name="wihT_sb")
            nc.sync.dma_start(out=wihT_sb[:], in_=wihT[:])
            b2_sb = cp.tile([2, 4 * H], F32, name="b2_sb")
            nc.sync.dma_start(out=b2_sb[:], in_=b2[:])
            sel23_sb = cp.tile([2, 3], F32, name="sel23_sb")
            nc.sync.dma_start(out=sel23_sb[:], in_=sel23[:])
            whhT_sb = cp.tile([H, 4 * H], F32, name="whhT_sb")
            nc.sync.dma_start(out=whhT_sb[:], in_=whhT[:])

            # NaN indicator (1.0 where NaN) per (t,b), per channel half
            ind = cp.tile([2 * T, BL], F16, name="ind")
            nc.vector.tensor_tensor(ind[0:T, :], obs_sb[0:T, :], obs_sb[0:T, :],
                                    AOP.not_equal)
            nc.vector.tensor_tensor(ind[T:2 * T, :], obs_sb[T:2 * T, :],
                                    obs_sb[T:2 * T, :], AOP.not_equal)
            # cleaned fp16 obs: NaN -> 0
            zeros = cp.tile([2 * T, BL], F16, name="zeros")
            nc.vector.memset(zeros[:], 0.0)
            obs16 = cp.tile([2 * T, BL], F16, name="obs16")
            nc.vector.tensor_copy(obs16[:], obs_sb[:])
            nc.vector.copy_predicated(obs16[:], ind[:].bitcast(mybir.dt.uint16),
                                      zeros[:])

            # fused input weights: psum_w = [W_x0; W_x1; b_x] (3, 512),
            # torch gate order i,f,g,o
            wt16 = cp.tile([H, 4 * H], F16, name="wt16")
            nc.vector.memset(wt16[:], 0.0)
            with tc.tile_pool(name="psum_prep", bufs=1, space="PSUM") as pp:
                psum_w = pp.tile([3, 4 * H], F32, name="psum_w")
                nc.tensor.matmul(psum_w[:], wemb3_sb[:], wihT_sb[:],
                                 start=True, stop=False)
                nc.tensor.matmul(psum_w[:], sel23_sb[:], b2_sb[:],
                                 start=False, stop=True)
                # W~ fp16 (128, 512) zero-padded; gate column order i,f,o,g
                nc.vector.tensor_copy(wt16[0:3, 0:2 * H], psum_w[:, 0:2 * H])
                nc.vector.tensor_copy(wt16[0:3, 2 * H:3 * H],
                                      psum_w[:, 3 * H:4 * H])
                nc.vector.tensor_copy(wt16[0:3, 3 * H:4 * H],
                                      psum_w[:, 2 * H:3 * H])
                nc.sync.dma_start(out=wt16[3:4, :], in_=mask16[:])

            # WhhT fp16, gate column order i,f,o,g
            whh16 = cp.tile([H, 4 * H], F16, name="whh16")
            nc.vector.tensor_copy(whh16[:, 0:2 * H], whhT_sb[:, 0:2 * H])
            nc.vector.tensor_copy(whh16[:, 2 * H:3 * H], whhT_sb[:, 3 * H:4 * H])
            nc.vector.tensor_copy(whh16[:, 3 * H:4 * H], whhT_sb[:, 2 * H:3 * H])

            # state (true scale)
            Hs = cp.tile([H, BL], F16, name="Hs")
            Cs = cp.tile([H, BL], F16, name="Cs")
            nc.vector.memset(Hs[:], 0.0)
            nc.vector.memset(Cs[:], 0.0)

            # x~ ping-pong buffers, zero-padded to K=128 once
            xbufs = []
            for i in range(2):
                xb = cp.tile([H, BLK * BL], F16, name=f"xb{i}")
                nc.vector.memset(xb[:], 0.0)
                xbufs.append(xb)

            # ---- steps (ragged: only the valid prefix width per step) ----
            with tc.tile_pool(name="psum_gates", bufs=2, space="PSUM") as gp:
                for tb in range(NBLK):
                    xb = xbufs[tb % 2]
                    t0 = tb * BLK
                    nc.sync.dma_start(out=xb[0:1, :], in_=obs16[t0:t0 + BLK, :])
                    nc.sync.dma_start(out=xb[1:2, :],
                                      in_=obs16[T + t0:T + t0 + BLK, :])
                    nc.sync.dma_start(out=xb[2:3, :], in_=ones16[:])
                    nc.sync.dma_start(out=xb[3:4, :], in_=ind[t0:t0 + BLK, :])

                    for dt_ in range(BLK):
                        t = t0 + dt_
                        W = widths[t]
                        nchunk = (W + C - 1) // C
                        for j in range(nchunk):
                            cw = min(C, W - j * C)
                            jc = slice(j * C, j * C + cw)
                            xoff = dt_ * BL + j * C
                            rhs_x = xb[:, xoff:xoff + cw]
                            g_ps = gp.tile([H, 4 * C], F32, name="g_ps")
                            for pb in range(4):
                                gs = slice(pb * C, pb * C + cw)
                                wsl = slice(pb * H, (pb + 1) * H)
                                nc.tensor.matmul(g_ps[:, gs], wt16[:, wsl],
                                                 rhs_x, start=True, stop=False)
                                nc.tensor.matmul(g_ps[:, gs], whh16[:, wsl],
                                                 Hs[:, jc], start=False,
                                                 stop=True)
                            sig = wp.tile([H, 3 * C], F16, name="sig")
                            if cw == C:
                                nc.scalar.activation(sig[:], g_ps[:, 0:3 * C],
                                                     ACTF.Sigmoid)
                            else:
                                sig_src = g_ps[:, 0:3 * C].rearrange(
                                    "p (g c) -> p g c", g=3)[:, :, 0:cw]
                                nc.scalar.activation(
                                    sig[:, 0:3 * cw].rearrange(
                                        "p (g c) -> p g c", g=3),
                                    sig_src, ACTF.Sigmoid)
                            tgg = wp.tile([H, C], F16, name="tgg")
                            nc.scalar.activation(tgg[:, 0:cw],
                                                 g_ps[:, 3 * C:3 * C + cw],
                                                 ACTF.Tanh)
                            ig = wp.tile([H, C], F16, name="ig")
                            nc.vector.tensor_tensor(ig[:, 0:cw],
                                                    sig[:, 0:cw], tgg[:, 0:cw],
                                                    AOP.mult)
                            fc = wp.tile([H, C], F16, name="fc")
                            nc.vector.tensor_tensor(fc[:, 0:cw],
                                                    sig[:, cw:2 * cw],
                                                    Cs[:, jc], AOP.mult)
                            nc.vector.tensor_tensor(Cs[:, jc], ig[:, 0:cw],
                                                    fc[:, 0:cw], AOP.add)
                            th = wp.tile([H, C], F16, name="th")
                            nc.scalar.activation(th[:, 0:cw], Cs[:, jc],
                                                 ACTF.Tanh)
                            nc.vector.tensor_tensor(Hs[:, jc],
                                                    sig[:, 2 * cw:3 * cw],
                                                    th[:, 0:cw], AOP.mult)

            # ---- output (fp32) ----
            hout = cp.tile([H, BL], F32, name="hout")
            nc.vector.tensor_copy(hout[:], Hs[:])
            nc.sync.dma_start(out=h_out[:], in_=hout[:])

    nc.compile()
    return nc


_CACHE = {}


def _plan(obs_traj):
    """Sort batch by ragged start (sharding permutation) and derive the
    per-step valid prefix width each core must process. Any width >= the
    true valid count is correct (masked lanes stay exactly 0)."""
    obs_traj = np.asarray(obs_traj)
    start = np.isnan(obs_traj[:, :, 0]).sum(0)          # (B,)
    perm = np.argsort(start, kind="stable")
    start_sorted = start[perm]
    ts = np.arange(T)
    vglob = np.searchsorted(start_sorted, ts, side="right")  # valid count
    w = np.ceil(vglob / N_CORES).astype(np.int64)
    w = np.minimum(BL, ((w + 7) // 8) * 8)
    w = np.maximum(w, 8)
    return perm, tuple(int(x) for x in w)


def _host_inputs(obs_traj, W_emb, b_emb, w_ih, w_hh, b_ih, b_hh, perm):
    f32 = np.float32
    wemb3 = np.concatenate(
        [np.asarray(W_emb, f32).T, np.asarray(b_emb, f32)[:, None]], axis=1
    )  # (64, 3)
    wihT = np.ascontiguousarray(np.asarray(w_ih, f32).T)      # (64, 512)
    whhT = np.ascontiguousarray(np.asarray(w_hh, f32).T)      # (128, 512)
    b2 = np.ascontiguousarray(
        np.stack([np.asarray(b_ih, f32), np.asarray(b_hh, f32)], axis=0)
    )  # (2, 512)
    sel23 = np.array([[0, 0, 1], [0, 0, 1]], f32)             # (2, 3)
    # mask row in device gate order [i, f, o, g]
    maskrow = np.zeros((1, 4 * H), np.float16)
    maskrow[0, 0:H] = -BIG          # i
    maskrow[0, 2 * H:3 * H] = -BIG  # o
    ones16 = np.ones((1, BLK * BL), np.float16)

    obs_traj = np.asarray(obs_traj)
    in_maps = []
    for k in range(N_CORES):
        sl = np.asarray(obs_traj[:, perm[k::N_CORES], :], f32)  # (T, BL, 2)
        obs_p = np.ascontiguousarray(
            sl.transpose(2, 0, 1).reshape(2 * T, BL)
        )  # (128, BL): row f*T + t
        in_maps.append({
            "obs_p": obs_p, "wemb3": wemb3, "wihT": wihT, "b2": b2,
            "sel23": sel23, "whhT": whhT, "mask16": maskrow, "ones16": ones16,
        })
    return in_maps


def kernel(obs_traj, W_emb, b_emb, w_ih, w_hh, b_ih, b_hh):
    perm, widths = _plan(obs_traj)
    if _CACHE.get("widths") != widths:
        _CACHE["nc"] = _build_program(widths)
        _CACHE["widths"] = widths
    nc = _CACHE["nc"]

    in_maps = _host_inputs(obs_traj, W_emb, b_emb, w_ih, w_hh, b_ih, b_hh,
                           perm)
    res = run_bass_kernel_spmd(nc, in_maps, list(range(N_CORES)))

    out = np.empty((1, B, H), np.float32)
    for k in range(N_CORES):
        out[0, perm[k::N_CORES], :] = res.results[k]["h_out"].T
    return out


# revision 10
# speedup vs baseline: 1.5673x; 1.0465x over previous
"""Trainium2 Bass kernel for the ragged-sequence LSTM encoder.

Math: masked LSTM over T=64 steps, B=16384, E=64, H=128. Reference:
  mask[t,b] = ~isnan(obs[t,b,0]); x = nan_to_num(obs)
  emb = x @ W_emb + b_emb
  gates = emb_t @ w_ih.T + h @ w_hh.T + (b_ih + b_hh);  i,f,g,o
  c' = f*c + i*g ; h' = o*tanh(c'); carry updated only where mask.

Kernel reformulation (exact up to fp rounding):
- The NaN prefix is monotone (invalid iff t < start[b]), so masked lanes
  have h=c=0 until their first valid step. Forcing i=0 and o=0 on masked
  lanes keeps h=c=0 exactly -> no select/blend ops needed. Done by an
  extra "nan indicator" input row with weight -BIG on the i/o gate
  blocks (sigmoid saturates to 0 exactly).
- Embedding folded into the input weights: W_x = W_emb @ w_ih.T,
  b_x = b_emb @ w_ih.T + b_ih + b_hh (both computed on device). Per-step
  input is x~_t = [x0, x1, 1, nan_ind, 0...] zero-padded to K=128 --
  the pad costs no PE cycles (cost ~ N columns) and keeps every matmul
  at the full (128,128) stationary shape; interleaving K=4 LDWEIGHTS
  with K=128 ones was measured to break PE pipelining (535 vs 216
  ns/matmul).
- Layout: hidden/gate dim on partitions, batch on free dim. Batch in 4
  chunks of 512 (PSUM bank per gate block). PSUM gate order [i,f,o,g]
  so one sigmoid ACTIVATE covers i,f,o and one tanh covers g.
- Per chunk per step:  psum = x~ @ W~ + H @ WhhT   (8 matmuls, K=128)
    sig = sigmoid(psum[:, :3C]); tg = tanh(psum[:, 3C:])
    ig = i*tg; fc = f*c; c' = ig + fc; th = tanh(c'); h' = o*th
  (4 fp16 tensor_tensor DVE ops at 2x mode + 3 ACT ops)
- Data parallel over batch: core k takes columns [k*2048,(k+1)*2048).
  Weights replicated.

Host code only reshapes/shards/transposes and supplies constants; all
FLOPs (weight fusion, NaN handling, LSTM) run on device.
"""

import sys
import numpy as np

for _p in ("/opt/trn_rl_repo", "/root/.axon_site/_ro/trn_rl_repo"):
    if _p not in sys.path:
        sys.path.insert(0, _p)

import concourse.bacc as bacc
import concourse.tile as tile
import concourse.mybir as mybir
from concourse.bass_utils import run_bass_kernel_spmd

F32 = mybir.dt.float32
F16 = mybir.dt.float16
AOP = mybir.AluOpType
ACTF = mybir.ActivationFunctionType

N_CORES = 8
T = 64
B = 16384
E = 64
H = 128
BL = B // N_CORES          # 2048 batch per core
C = 512                    # batch chunk (one PSUM bank per gate block)
NCHUNK = BL // C           # 4
BLK = 8                    # time steps per streamed x~ block
NBLK = T // BLK
BIG = 30000.0


def _build_program(widths):
    nc = bacc.Bacc()

    obs_p = nc.dram_tensor("obs_p", [2 * T, BL], F32, kind="ExternalInput")
    wemb3 = nc.dram_tensor("wemb3", [E, 3], F32, kind="ExternalInput")
    wihT = nc.dram_tensor("wihT", [E, 4 * H], F32, kind="ExternalInput")
    b2 = nc.dram_tensor("b2", [2, 4 * H], F32, kind="ExternalInput")
    sel23 = nc.dram_tensor("sel23", [2, 3], F32, kind="ExternalInput")
    whhT = nc.dram_tensor("whhT", [H, 4 * H], F32, kind="ExternalInput")
    mask16 = nc.dram_tensor("mask16", [1, 4 * H], F16, kind="ExternalInput")
    ones16 = nc.dram_tensor("ones16", [1, BLK * BL], F16, kind="ExternalInput")
    h_out = nc.dram_tensor("h_out", [H, BL], F32, kind="ExternalOutput")

    with tile.TileContext(nc) as tc:
        with (
            tc.tile_pool(name="const", bufs=1) as cp,
            tc.tile_pool(name="work", bufs=6) as wp,
        ):
            # ---- one-time prep ----
            obs_sb = cp.tile([2 * T, BL], F32, name="obs_sb")
            nc.sync.dma_start(out=obs_sb[:], in_=obs_p[:])
            wemb3_sb = cp.tile([E, 3], F32, name="wemb3_sb")
            nc.sync.dma_start(out=wemb3_sb[:], in_=wemb3[:])
            wihT_sb = cp.tile([E, 4 * H], F32, name="wihT_sb")
            nc.sync.dma_start(out=wihT_sb[:], in_=wihT[:])
            b2_sb = cp.tile([2, 4 * H], F32, name="b2_sb")
            nc.sync.dma_start(out=b2_sb[:], in_=b2[:])
            sel23_sb = cp.tile([2, 3], F32, name="sel23_sb")
            nc.sync.dma_start(out=sel23_sb[:], in_=sel23[:])
            whhT_sb = cp.tile([H, 4 * H], F32, name="whhT_sb")
            nc.sync.dma_start(out=whhT_sb[:], in_=whhT[:])

            # NaN indicator (1.0 where NaN) per (t,b), per channel half
            ind = cp.tile([2 * T, BL], F16, name="ind")
            nc.vector.tensor_tensor(ind[0:T, :], obs_sb[0:T, :], obs_sb[0:T, :],
                                    AOP.not_equal)
            nc.vector.tensor_tensor(ind[T:2 * T, :], obs_sb[T:2 * T, :],
                                    obs_sb[T:2 * T, :], AOP.not_equal)
            # cleaned fp16 obs: NaN -> 0
            zeros = cp.tile([2 * T, BL], F16, name="zeros")
            nc.vector.memset(zeros[:], 0.0)
            obs16 = cp.tile([2 * T, BL], F16, name="obs16")
            nc.vector.tensor_copy(obs16[:], obs_sb[:])
            nc.vector.copy_predicated(obs16[:], ind[:].bitcast(mybir.dt.uint16),
                                      zeros[:])

            # fused input weights: psum_w = [W_x0; W_x1; b_x] (3, 512),
            # torch gate order i,f,g,o
            wt16 = cp.tile([H, 4 * H], F16, name="wt16")
            nc.vector.memset(wt16[:], 0.0)
            with tc.tile_pool(name="psum_prep", bufs=1, space="PSUM") as pp:
                psum_w = pp.tile([3, 4 * H], F32, name="psum_w")
                nc.tensor.matmul(psum_w[:], wemb3_sb[:], wihT_sb[:],
                                 start=True, stop=False)
                nc.tensor.matmul(psum_w[:], sel23_sb[:], b2_sb[:],
                                 start=False, stop=True)
                # W~ fp16 (128, 512) zero-padded; gate column order i,f,o,g
                nc.vector.tensor_copy(wt16[0:3, 0:2 * H], psum_w[:, 0:2 * H])
                nc.vector.tensor_copy(wt16[0:3, 2 * H:3 * H],
                                      psum_w[:, 3 * H:4 * H])
                nc.vector.tensor_copy(wt16[0:3, 3 * H:4 * H],
                                      psum_w[:, 2 * H:3 * H])
                nc.sync.dma_start(out=wt16[3:4, :], in_=mask16[:])

            # WhhT fp16, gate column order i,f,o,g
            whh16 = cp.tile([H, 4 * H], F16, name="whh16")
            nc.vector.tensor_copy(whh16[:, 0:2 * H], whhT_sb[:, 0:2 * H])
            nc.vector.tensor_copy(whh16[:, 2 * H:3 * H], whhT_sb[:, 3 * H:4 * H])
            nc.vector.tensor_copy(whh16[:, 3 * H:4 * H], whhT_sb[:, 2 * H:3 * H])

            # state (true scale)
            Hs = cp.tile([H, BL], F16, name="Hs")
            Cs = cp.tile([H, BL], F16, name="Cs")
            nc.vector.memset(Hs[:], 0.0)
            nc.vector.memset(Cs[:], 0.0)

            # x~ ping-pong buffers, zero-padded to K=128 once
            xbufs = []
            for i in range(2):
                xb = cp.tile([H, BLK * BL], F16, name=f"xb{i}")
                nc.vector.memset(xb[:], 0.0)
                xbufs.append(xb)

            # ---- steps (ragged: only the valid prefix width per step) ----
            with tc.tile_pool(name="psum_gates", bufs=2, space="PSUM") as gp:
                for tb in range(NBLK):
                    xb = xbufs[tb % 2]
                    t0 = tb * BLK
                    nc.sync.dma_start(out=xb[0:1, :], in_=obs16[t0:t0 + BLK, :])
                    nc.sync.dma_start(out=xb[1:2, :],
                                      in_=obs16[T + t0:T + t0 + BLK, :])
                    nc.sync.dma_start(out=xb[2:3, :], in_=ones16[:])
                    nc.sync.dma_start(out=xb[3:4, :], in_=ind[t0:t0 + BLK, :])

                    for dt_ in range(BLK):
                        t = t0 + dt_
                        W = widths[t]
                        nchunk = (W + C - 1) // C
                        for j in range(nchunk):
                            cw = min(C, W - j * C)
                            jc = slice(j * C, j * C + cw)
                            xoff = dt_ * BL + j * C
                            rhs_x = xb[:, xoff:xoff + cw]
                            g_ps = gp.tile([H, 4 * C], F32, name="g_ps")
                            for pb in range(4):
                                gs = slice(pb * C, pb * C + cw)
                                nc.tensor.matmul(g_ps[:, gs],
                                                 wt16[:, pb * H:(pb + 1) * H],
                                                 rhs_x, start=True, stop=False)
                            for pb in range(4):
                                gs = slice(pb * C, pb * C + cw)
                                nc.tensor.matmul(g_ps[:, gs],
                                                 whh16[:, pb * H:(pb + 1) * H],
                                                 Hs[:, jc], start=False,
                                                 stop=True)
                            sig = wp.tile([H, 3 * C], F16, name="sig")
                            if cw == C:
                                nc.scalar.activation(sig[:], g_ps[:, 0:3 * C],
                                                     ACTF.Sigmoid)
                            else:
                                sig_src = g_ps[:, 0:3 * C].rearrange(
                                    "p (g c) -> p g c", g=3)[:, :, 0:cw]
                                nc.scalar.activation(
                                    sig[:, 0:3 * cw].rearrange(
                                        "p (g c) -> p g c", g=3),
                                    sig_src, ACTF.Sigmoid)
                            tgg = wp.tile([H, C], F16, name="tgg")
                            nc.scalar.activation(tgg[:, 0:cw],
                                                 g_ps[:, 3 * C:3 * C + cw],
                                                 ACTF.Tanh)
                            ig = wp.tile([H, C], F16, name="ig")
                            nc.vector.tensor_tensor(ig[:, 0:cw],
                                                    sig[:, 0:cw], tgg[:, 0:cw],
                                                    AOP.mult)
                            fc = wp.tile([H, C], F16, name="fc")
                            nc.vector.tensor_tensor(fc[:, 0:cw],
                                                    sig[:, cw:2 * cw],
                                                    Cs[:, jc], AOP.mult)
                            nc.vector.tensor_tensor(Cs[:, jc], ig[:, 0:cw],
                                                    fc[:, 0:cw], AOP.add)
                            th = wp.tile([H, C], F16, name="th")
                            nc.scalar.activation(th[:, 0:cw], Cs[:, jc],
                                                 ACTF.Tanh)
                            nc.vector.tensor_tensor(Hs[:, jc],
                                                    sig[:, 2 * cw:3 * cw],
                                                    th[:, 0:cw], AOP.mult)

            # ---- output (fp32) ----
            hout = cp.tile([H, BL], F32, name="hout")
            nc.vector.tensor_copy(hout[:], Hs[:])
            nc.sync.dma_start(out=h_out[:], in_=hout[:])

    nc.compile()
    return nc


_CACHE = {}


def _plan(obs_traj):
    """Sort batch by ragged start (sharding permutation) and derive the
    per-step valid prefix width each core must process. Any width >= the
    true valid count is correct (masked lanes stay exactly 0)."""
    obs_traj = np.asarray(obs_traj)
    start = np.isnan(obs_traj[:, :, 0]).sum(0)          # (B,)
    perm = np.argsort(start, kind="stable")
    start_sorted = start[perm]
    ts = np.arange(T)
    vglob = np.searchsorted(start_sorted, ts, side="right")  # valid count
    w = np.ceil(vglob / N_CORES).astype(np.int64)
    w = np.minimum(BL, ((w + 7) // 8) * 8)
    w = np.maximum(w, 8)
    return perm, tuple(int(x) for x in w)


def _host_inputs(obs_traj, W_emb, b_emb, w_ih, w_hh, b_ih, b_hh, perm):
    f32 = np.float32
    wemb3 = np.concatenate(
        [np.asarray(W_emb, f32).T, np.asarray(b_emb, f32)[:, None]], axis=1
    )  # (64, 3)
    wihT = np.ascontiguousarray(np.asarray(w_ih, f32).T)      # (64, 512)
    whhT = np.ascontiguousarray(np.asarray(w_hh, f32).T)      # (128, 512)
    b2 = np.ascontiguousarray(
        np.stack([np.asarray(b_ih, f32), np.asarray(b_hh, f32)], axis=0)
    )  # (2, 512)
    sel23 = np.array([[0, 0, 1], [0, 0, 1]], f32)             # (2, 3)
    # mask row in device gate order [i, f, o, g]
    maskrow = np.zeros((1, 4 * H), np.float16)
    maskrow[0, 0:H] = -BIG          # i
    maskrow[0, 2 * H:3 * H] = -BIG  # o
    ones16 = np.ones((1, BLK * BL), np.float16)

    obs_traj = np.asarray(obs_traj)
    in_maps = []
    for k in range(N_CORES):
        sl = np.asarray(obs_traj[:, perm[k::N_CORES], :], f32)  # (T, BL, 2)
        obs_p = np.ascontiguousarray(
            sl.transpose(2, 0, 1).reshape(2 * T, BL)
        )  # (128, BL): row f*T + t
        in_maps.append({
            "obs_p": obs_p, "wemb3": wemb3, "wihT": wihT, "b2": b2,
            "sel23": sel23, "whhT": whhT, "mask16": maskrow, "ones16": ones16,
        })
    return in_maps


def kernel(obs_traj, W_emb, b_emb, w_ih, w_hh, b_ih, b_hh):
    perm, widths = _plan(obs_traj)
    if _CACHE.get("widths") != widths:
        _CACHE["nc"] = _build_program(widths)
        _CACHE["widths"] = widths
    nc = _CACHE["nc"]

    in_maps = _host_inputs(obs_traj, W_emb, b_emb, w_ih, w_hh, b_ih, b_hh,
                           perm)
    res = run_bass_kernel_spmd(nc, in_maps, list(range(N_CORES)))

    out = np.empty((1, B, H), np.float32)
    for k in range(N_CORES):
        out[0, perm[k::N_CORES], :] = res.results[k]["h_out"].T
    return out


# revision 11
# speedup vs baseline: 1.6742x; 1.0682x over previous
"""Trainium2 Bass kernel for the ragged-sequence LSTM encoder.

Math: masked LSTM over T=64 steps, B=16384, E=64, H=128. Reference:
  mask[t,b] = ~isnan(obs[t,b,0]); x = nan_to_num(obs)
  emb = x @ W_emb + b_emb
  gates = emb_t @ w_ih.T + h @ w_hh.T + (b_ih + b_hh);  i,f,g,o
  c' = f*c + i*g ; h' = o*tanh(c'); carry updated only where mask.

Kernel reformulation (exact up to fp rounding):
- The NaN prefix is monotone (invalid iff t < start[b]), so masked lanes
  have h=c=0 until their first valid step. Forcing i=0 and o=0 on masked
  lanes keeps h=c=0 exactly -> no select/blend ops needed. Done by an
  extra "nan indicator" input row with weight -BIG on the i/o gate
  blocks (sigmoid saturates to 0 exactly).
- Embedding folded into the input weights: W_x = W_emb @ w_ih.T,
  b_x = b_emb @ w_ih.T + b_ih + b_hh (both computed on device). Per-step
  input is x~_t = [x0, x1, 1, nan_ind, 0...] zero-padded to K=128 --
  the pad costs no PE cycles (cost ~ N columns) and keeps every matmul
  at the full (128,128) stationary shape; interleaving K=4 LDWEIGHTS
  with K=128 ones was measured to break PE pipelining (535 vs 216
  ns/matmul).
- Layout: hidden/gate dim on partitions, batch on free dim. Batch in 4
  chunks of 512 (PSUM bank per gate block). PSUM gate order [i,f,o,g]
  so one sigmoid ACTIVATE covers i,f,o and one tanh covers g.
- Per chunk per step:  psum = x~ @ W~ + H @ WhhT   (8 matmuls, K=128)
    sig = sigmoid(psum[:, :3C]); tg = tanh(psum[:, 3C:])
    ig = i*tg; fc = f*c; c' = ig + fc; th = tanh(c'); h' = o*th
  (4 fp16 tensor_tensor DVE ops at 2x mode + 3 ACT ops)
- Data parallel over batch: core k takes columns [k*2048,(k+1)*2048).
  Weights replicated.

Host code only reshapes/shards/transposes and supplies constants; all
FLOPs (weight fusion, NaN handling, LSTM) run on device.
"""

import sys
import numpy as np

for _p in ("/opt/trn_rl_repo", "/root/.axon_site/_ro/trn_rl_repo"):
    if _p not in sys.path:
        sys.path.insert(0, _p)

import concourse.bacc as bacc
import concourse.tile as tile
import concourse.mybir as mybir
from concourse.bass_utils import run_bass_kernel_spmd

F32 = mybir.dt.float32
F16 = mybir.dt.float16
AOP = mybir.AluOpType
ACTF = mybir.ActivationFunctionType

N_CORES = 8
T = 64
B = 16384
E = 64
H = 128
BL = B // N_CORES          # 2048 batch per core
C = 512                    # batch chunk (one PSUM bank per gate block)
NCHUNK = BL // C           # 4
BLK = 8                    # time steps per streamed x~ block
NBLK = T // BLK
BIG = 30000.0


def _build_program(widths):
    nc = bacc.Bacc()

    obs_p = nc.dram_tensor("obs_p", [2 * T, BL], F32, kind="ExternalInput")
    wemb3 = nc.dram_tensor("wemb3", [E, 3], F32, kind="ExternalInput")
    wihT = nc.dram_tensor("wihT", [E, 4 * H], F32, kind="ExternalInput")
    b2 = nc.dram_tensor("b2", [2, 4 * H], F32, kind="ExternalInput")
    sel23 = nc.dram_tensor("sel23", [2, 3], F32, kind="ExternalInput")
    whhT = nc.dram_tensor("whhT", [H, 4 * H], F32, kind="ExternalInput")
    mask16 = nc.dram_tensor("mask16", [1, 4 * H], F16, kind="ExternalInput")
    ones16 = nc.dram_tensor("ones16", [1, BLK * BL], F16, kind="ExternalInput")
    h_out = nc.dram_tensor("h_out", [H, BL], F32, kind="ExternalOutput")

    with tile.TileContext(nc) as tc:
        with (
            tc.tile_pool(name="const", bufs=1) as cp,
            tc.tile_pool(name="work", bufs=6) as wp,
        ):
            # ---- one-time prep ----
            obs_sb = cp.tile([2 * T, BL], F32, name="obs_sb")
            nc.sync.dma_start(out=obs_sb[:], in_=obs_p[:])
            wemb3_sb = cp.tile([E, 3], F32, name="wemb3_sb")
            nc.sync.dma_start(out=wemb3_sb[:], in_=wemb3[:])
            wihT_sb = cp.tile([E, 4 * H], F32, name="wihT_sb")
            nc.sync.dma_start(out=wihT_sb[:], in_=wihT[:])
            b2_sb = cp.tile([2, 4 * H], F32, name="b2_sb")
            nc.sync.dma_start(out=b2_sb[:], in_=b2[:])
            sel23_sb = cp.tile([2, 3], F32, name="sel23_sb")
            nc.sync.dma_start(out=sel23_sb[:], in_=sel23[:])
            whhT_sb = cp.tile([H, 4 * H], F32, name="whhT_sb")
            nc.sync.dma_start(out=whhT_sb[:], in_=whhT[:])

            # NaN indicator (1.0 where NaN) per (t,b), per channel half
            ind = cp.tile([2 * T, BL], F16, name="ind")
            nc.vector.tensor_tensor(ind[0:T, :], obs_sb[0:T, :], obs_sb[0:T, :],
                                    AOP.not_equal)
            nc.vector.tensor_tensor(ind[T:2 * T, :], obs_sb[T:2 * T, :],
                                    obs_sb[T:2 * T, :], AOP.not_equal)
            # cleaned fp16 obs: NaN -> 0
            zeros = cp.tile([2 * T, BL], F16, name="zeros")
            nc.gpsimd.memset(zeros[:], 0.0)
            obs16 = cp.tile([2 * T, BL], F16, name="obs16")
            nc.vector.tensor_copy(obs16[:], obs_sb[:])
            nc.vector.copy_predicated(obs16[:], ind[:].bitcast(mybir.dt.uint16),
                                      zeros[:])

            # fused input weights: psum_w = [W_x0; W_x1; b_x] (3, 512),
            # torch gate order i,f,g,o
            wt16 = cp.tile([H, 4 * H], F16, name="wt16")
            nc.vector.memset(wt16[:], 0.0)
            with tc.tile_pool(name="psum_prep", bufs=1, space="PSUM") as pp:
                psum_w = pp.tile([3, 4 * H], F32, name="psum_w")
                nc.tensor.matmul(psum_w[:], wemb3_sb[:], wihT_sb[:],
                                 start=True, stop=False)
                nc.tensor.matmul(psum_w[:], sel23_sb[:], b2_sb[:],
                                 start=False, stop=True)
                # W~ fp16 (128, 512) zero-padded; gate column order i,f,o,g
                nc.vector.tensor_copy(wt16[0:3, 0:2 * H], psum_w[:, 0:2 * H])
                nc.vector.tensor_copy(wt16[0:3, 2 * H:3 * H],
                                      psum_w[:, 3 * H:4 * H])
                nc.vector.tensor_copy(wt16[0:3, 3 * H:4 * H],
                                      psum_w[:, 2 * H:3 * H])
                nc.sync.dma_start(out=wt16[3:4, :], in_=mask16[:])

            # WhhT fp16, gate column order i,f,o,g
            whh16 = cp.tile([H, 4 * H], F16, name="whh16")
            nc.vector.tensor_copy(whh16[:, 0:2 * H], whhT_sb[:, 0:2 * H])
            nc.vector.tensor_copy(whh16[:, 2 * H:3 * H], whhT_sb[:, 3 * H:4 * H])
            nc.vector.tensor_copy(whh16[:, 3 * H:4 * H], whhT_sb[:, 2 * H:3 * H])

            # state (true scale)
            Hs = cp.tile([H, BL], F16, name="Hs")
            Cs = cp.tile([H, BL], F16, name="Cs")
            nc.vector.memset(Hs[:], 0.0)
            nc.vector.memset(Cs[:], 0.0)

            # x~ ping-pong buffers, zero-padded to K=128 once
            xbufs = []
            for i in range(2):
                xb = cp.tile([H, BLK * BL], F16, name=f"xb{i}")
                nc.gpsimd.memset(xb[:], 0.0)
                xbufs.append(xb)

            # ---- steps (ragged: only the valid prefix width per step) ----
            with tc.tile_pool(name="psum_gates", bufs=2, space="PSUM") as gp:
                for tb in range(NBLK):
                    xb = xbufs[tb % 2]
                    t0 = tb * BLK
                    nc.sync.dma_start(out=xb[0:1, :], in_=obs16[t0:t0 + BLK, :])
                    nc.sync.dma_start(out=xb[1:2, :],
                                      in_=obs16[T + t0:T + t0 + BLK, :])
                    nc.sync.dma_start(out=xb[2:3, :], in_=ones16[:])
                    nc.sync.dma_start(out=xb[3:4, :], in_=ind[t0:t0 + BLK, :])

                    for dt_ in range(BLK):
                        t = t0 + dt_
                        W = widths[t]
                        cwt = min(C, max(64, ((W // 4 + 7) // 8) * 8))
                        nchunk = (W + cwt - 1) // cwt
                        for j in range(nchunk):
                            cw = min(cwt, W - j * cwt)
                            jc = slice(j * cwt, j * cwt + cw)
                            xoff = dt_ * BL + j * cwt
                            rhs_x = xb[:, xoff:xoff + cw]
                            g_ps = gp.tile([H, 4 * C], F32, name="g_ps")
                            for pb in range(4):
                                gs = slice(pb * C, pb * C + cw)
                                nc.tensor.matmul(g_ps[:, gs],
                                                 wt16[:, pb * H:(pb + 1) * H],
                                                 rhs_x, start=True, stop=False)
                            for pb in range(4):
                                gs = slice(pb * C, pb * C + cw)
                                nc.tensor.matmul(g_ps[:, gs],
                                                 whh16[:, pb * H:(pb + 1) * H],
                                                 Hs[:, jc], start=False,
                                                 stop=True)
                            sig = wp.tile([H, 3 * C], F16, name="sig")
                            if cw == C:
                                nc.scalar.activation(sig[:], g_ps[:, 0:3 * C],
                                                     ACTF.Sigmoid)
                            else:
                                sig_src = g_ps[:, 0:3 * C].rearrange(
                                    "p (g c) -> p g c", g=3)[:, :, 0:cw]
                                nc.scalar.activation(
                                    sig[:, 0:3 * cw].rearrange(
                                        "p (g c) -> p g c", g=3),
                                    sig_src, ACTF.Sigmoid)
                            tgg = wp.tile([H, C], F16, name="tgg")
                            nc.scalar.activation(tgg[:, 0:cw],
                                                 g_ps[:, 3 * C:3 * C + cw],
                                                 ACTF.Tanh)
                            ig = wp.tile([H, C], F16, name="ig")
                            nc.vector.tensor_tensor(ig[:, 0:cw],
                                                    sig[:, 0:cw], tgg[:, 0:cw],
                                                    AOP.mult)
                            fc = wp.tile([H, C], F16, name="fc")
                            nc.vector.tensor_tensor(fc[:, 0:cw],
                                                    sig[:, cw:2 * cw],
                                                    Cs[:, jc], AOP.mult)
                            nc.vector.tensor_tensor(Cs[:, jc], ig[:, 0:cw],
                                                    fc[:, 0:cw], AOP.add)
                            th = wp.tile([H, C], F16, name="th")
                            nc.scalar.activation(th[:, 0:cw], Cs[:, jc],
                                                 ACTF.Tanh)
                            nc.vector.tensor_tensor(Hs[:, jc],
                                                    sig[:, 2 * cw:3 * cw],
                                                    th[:, 0:cw], AOP.mult)

            # ---- output (fp32) ----
            hout = cp.tile([H, BL], F32, name="hout")
            nc.vector.tensor_copy(hout[:], Hs[:])
            nc.sync.dma_start(out=h_out[:], in_=hout[:])

    nc.compile()
    return nc


_CACHE = {}


def _plan(obs_traj):
    """Sort batch by ragged start (sharding permutation) and derive the
    per-step valid prefix width each core must process. Any width >= the
    true valid count is correct (masked lanes stay exactly 0)."""
    obs_traj = np.asarray(obs_traj)
    start = np.isnan(obs_traj[:, :, 0]).sum(0)          # (B,)
    perm = np.argsort(start, kind="stable")
    start_sorted = start[perm]
    ts = np.arange(T)
    vglob = np.searchsorted(start_sorted, ts, side="right")  # valid count
    w = np.ceil(vglob / N_CORES).astype(np.int64)
    w = np.minimum(BL, ((w + 7) // 8) * 8)
    w = np.maximum(w, 8)
    return perm, tuple(int(x) for x in w)


def _host_inputs(obs_traj, W_emb, b_emb, w_ih, w_hh, b_ih, b_hh, perm):
    f32 = np.float32
    wemb3 = np.concatenate(
        [np.asarray(W_emb, f32).T, np.asarray(b_emb, f32)[:, None]], axis=1
    )  # (64, 3)
    wihT = np.ascontiguousarray(np.asarray(w_ih, f32).T)      # (64, 512)
    whhT = np.ascontiguousarray(np.asarray(w_hh, f32).T)      # (128, 512)
    b2 = np.ascontiguousarray(
        np.stack([np.asarray(b_ih, f32), np.asarray(b_hh, f32)], axis=0)
    )  # (2, 512)
    sel23 = np.array([[0, 0, 1], [0, 0, 1]], f32)             # (2, 3)
    # mask row in device gate order [i, f, o, g]
    maskrow = np.zeros((1, 4 * H), np.float16)
    maskrow[0, 0:H] = -BIG          # i
    maskrow[0, 2 * H:3 * H] = -BIG  # o
    ones16 = np.ones((1, BLK * BL), np.float16)

    obs_traj = np.asarray(obs_traj)
    in_maps = []
    for k in range(N_CORES):
        sl = np.asarray(obs_traj[:, perm[k::N_CORES], :], f32)  # (T, BL, 2)
        obs_p = np.ascontiguousarray(
            sl.transpose(2, 0, 1).reshape(2 * T, BL)
        )  # (128, BL): row f*T + t
        in_maps.append({
            "obs_p": obs_p, "wemb3": wemb3, "wihT": wihT, "b2": b2,
            "sel23": sel23, "whhT": whhT, "mask16": maskrow, "ones16": ones16,
        })
    return in_maps


def kernel(obs_traj, W_emb, b_emb, w_ih, w_hh, b_ih, b_hh):
    perm, widths = _plan(obs_traj)
    if _CACHE.get("widths") != widths:
        _CACHE["nc"] = _build_program(widths)
        _CACHE["widths"] = widths
    nc = _CACHE["nc"]

    in_maps = _host_inputs(obs_traj, W_emb, b_emb, w_ih, w_hh, b_ih, b_hh,
                           perm)
    res = run_bass_kernel_spmd(nc, in_maps, list(range(N_CORES)))

    out = np.empty((1, B, H), np.float32)
    for k in range(N_CORES):
        out[0, perm[k::N_CORES], :] = res.results[k]["h_out"].T
    return out
